# revision 10
# baseline (speedup 1.0000x reference)
"""Trainium2 Bass kernel for 2D MHSA with relative position logits.

Problem (per batch element b of 8, one NeuronCore each — pure data parallel):
    qkv = w_qkv @ featuremap[b]            # [3072, 1024]
    per head n (8 heads, d=128):
      logits = (q*s) @ k^T + relpos(q*s)   # [1024, 1024]
      out[n] = softmax(logits) @ v         # [1024, 128]

Layout strategy (everything chains without transposes after the initial
on-device transpose of w_qkv):
  - q, k produced as [d, x] tiles (d = partitions), v produced transposed
    as [y, d] tiles by swapping matmul operand roles in the projection.
  - logits computed transposed [y, x]; softmax denominator via a ones-
    matmul; 1/Z folded into the output eviction; O^T = v.T-accum directly
    matches the required output layout [n*d, h*w].
  - rel-pos logits: rw[x,(H,W)] = Lw[x, W-w(x)+31] (indep of H), rh
    likewise. Gather matrices G[b,x] = L[x, b-w(x)+31] are built with 64
    shifted-slice matmuls against relT, then folded into the logits PSUM
    accumulation as one K=64 matmul against a constant one-hot matrix.
  - exp() happens on the Scalar engine during PSUM eviction; no max
    subtraction (logits are bounded by ~±2 here; validated vs reference).

All matmul operands are float32r (full fp32 storage; PE fast mode).
"""

import os
import sys

for _p in ("/opt/trn_rl_repo", "/root/.axon_site/_ro/trn_rl_repo"):
    if os.path.isdir(_p) and _p not in sys.path:
        sys.path.append(_p)

import numpy as np

import concourse.bass as bass
import concourse.tile as tile
from concourse import bacc, mybir

F32R = mybir.dt.float32r
F32 = mybir.dt.float32

B = 8          # batch == number of cores
NH = 8         # heads
D = 128        # head dim
H = 32
W = 32
HW = H * W     # 1024 positions
C = 512        # channels
O3 = 3 * NH * D  # 3072 qkv rows
SCALE = D ** -0.5


def build_nc(num_devices: int = B):
    nc = bacc.Bacc("TRN2", target_bir_lowering=False, debug=False,
                   num_devices=num_devices)

    f_d = nc.dram_tensor("f", [C, HW], F32R, kind="ExternalInput")
    w_d = nc.dram_tensor("w", [O3, C], F32R, kind="ExternalInput")
    relh_d = nc.dram_tensor("relh", [2 * H, D], F32R, kind="ExternalInput")
    relw_d = nc.dram_tensor("relw", [2 * W, D], F32R, kind="ExternalInput")
    ident_d = nc.dram_tensor("ident", [128, 128], F32R, kind="ExternalInput")
    onehot_d = nc.dram_tensor("onehot", [64, HW], F32R, kind="ExternalInput")
    ones_d = nc.dram_tensor("ones", [128, 128], F32R, kind="ExternalInput")
    out_d = nc.dram_tensor("out", [NH * D, HW], F32, kind="ExternalOutput")

    with tile.TileContext(nc) as tc:
        _trace(nc, tc, f_d, w_d, relh_d, relw_d, ident_d, onehot_d, ones_d,
               out_d)
    nc.compile()
    return nc


def _trace(nc, tc, f_d, w_d, relh_d, relw_d, ident_d, onehot_d, ones_d, out_d):
    from contextlib import ExitStack

    with ExitStack() as outer:
        # ---- persistent SBUF pools -------------------------------------
        big = outer.enter_context(tc.tile_pool(name="big", bufs=1))
        q_all = big.tile([128, NH * HW], F32R, tag="q_all", name="q_all")
        k_all = big.tile([128, NH * HW], F32R, tag="k_all", name="k_all")
        v_all = big.tile([128, NH * HW], F32R, tag="v_all", name="v_all")

        cst = outer.enter_context(tc.tile_pool(name="cst", bufs=1))
        ident = cst.tile([128, 128], F32R, tag="ident", name="ident")
        onehot = cst.tile([64, HW], F32R, tag="onehot", name="onehot")
        ones = cst.tile([128, 128], F32R, tag="ones", name="ones")
        relwT = cst.tile([128, 64], F32R, tag="relwT", name="relwT")
        relhT = cst.tile([128, 64], F32R, tag="relhT", name="relhT")

        nc.sync.dma_start(ident[:], ident_d[:])
        nc.sync.dma_start(onehot[:], onehot_d[:])
        nc.sync.dma_start(ones[:], ones_d[:])

        # ---- phase 1: transpose w, project q/k/v -----------------------
        with ExitStack() as ph1:
            fp = ph1.enter_context(tc.tile_pool(name="fp", bufs=1))
            wtp = ph1.enter_context(tc.tile_pool(name="wtp", bufs=1))
            wst = ph1.enter_context(tc.tile_pool(name="wst", bufs=6))
            rst = ph1.enter_context(tc.tile_pool(name="rst", bufs=2))
            ps_tr = ph1.enter_context(
                tc.tile_pool(name="ps_tr", bufs=2, space=bass.MemorySpace.PSUM))
            ps_pj = ph1.enter_context(
                tc.tile_pool(name="ps_pj", bufs=4, space=bass.MemorySpace.PSUM))

            f_sb = []
            for i in range(4):
                ft = fp.tile([128, HW], F32R, tag=f"f{i}", name=f"f{i}")
                nc.sync.dma_start(ft[:], f_d[i * 128:(i + 1) * 128, :])
                f_sb.append(ft)

            # rel transposes: [63, 128] -> [128, 63] (padded to 64 rows for
            # the f32r transpose mode; col 63 of relT is never read)
            for (rd, rT, nm) in ((relw_d, relwT, "rw"), (relh_d, relhT, "rh")):
                rs = rst.tile([64, 128], F32R, tag="rs", name=f"rs_{nm}")
                nc.sync.dma_start(rs[:], rd[:])
                pt = ps_tr.tile([128, 64], F32R, tag="ptr_r", name=f"ptr_{nm}")
                nc.tensor.transpose(pt[:], rs[:], ident[0:64, 0:64])
                nc.vector.tensor_copy(rT[:], pt[:])

            # w transpose: wT[cb] [128, 3072], wT[cb][:, o] = w[o, cb*128+p]
            wT = []
            for cb in range(4):
                t = wtp.tile([128, O3], F32R, tag=f"wT{cb}", name=f"wT{cb}")
                wT.append(t)
            for og in range(6):
                wtiles = []
                for oi in range(4):
                    ob = og * 4 + oi
                    wt_ = wst.tile([128, C], F32R, tag="wst", name=f"w{ob}")
                    nc.sync.dma_start(wt_[:], w_d[ob * 128:(ob + 1) * 128, :])
                    wtiles.append(wt_)
                for cb in range(4):
                    pt = ps_tr.tile([128, 512], F32R, tag="ptr_w",
                                    name=f"ptw{og}_{cb}")
                    for oi in range(4):
                        nc.tensor.transpose(
                            pt[:, oi * 128:(oi + 1) * 128],
                            wtiles[oi][:, cb * 128:(cb + 1) * 128],
                            ident[:])
                    nc.vector.tensor_copy(
                        wT[cb][:, og * 512:(og + 1) * 512], pt[:])

            # q/k projection: out[o_blk(128), x] ; o_blk 0-7 q heads, 8-15 k
            for ob in range(16):
                for ch in range(2):
                    ps = ps_pj.tile([128, 512], F32, tag="pj", name=f"pjqk{ob}_{ch}")
                    for cb in range(4):
                        nc.tensor.matmul(
                            ps[:],
                            wT[cb][:, ob * 128:(ob + 1) * 128],
                            f_sb[cb][:, ch * 512:(ch + 1) * 512],
                            start=(cb == 0), stop=(cb == 3))
                    dst = (q_all if ob < 8 else k_all)
                    col = (ob % 8) * HW + ch * 512
                    if ob < 8:
                        nc.vector.tensor_scalar_mul(
                            dst[:, col:col + 512], ps[:], SCALE)
                    else:
                        nc.vector.tensor_copy(dst[:, col:col + 512], ps[:])

            # v projection, transposed: out[y_blk(128), o_v] with
            # lhsT = f tile, rhs = wT v-columns
            for yb in range(8):
                for oc in range(2):
                    ps = ps_pj.tile([128, 512], F32, tag="pj", name=f"pjv{yb}_{oc}")
                    for cb in range(4):
                        nc.tensor.matmul(
                            ps[:],
                            f_sb[cb][:, yb * 128:(yb + 1) * 128],
                            wT[cb][:, 2048 + oc * 512:2048 + (oc + 1) * 512],
                            start=(cb == 0), stop=(cb == 3))
                    nc.vector.tensor_copy(
                        v_all[:, yb * HW + oc * 512:yb * HW + (oc + 1) * 512],
                        ps[:])

        # ---- G gather matrices -----------------------------------------
        # G[b, x] (b<32):  Lw[x, b - w(x) + 31] ; G[32+b, x]: Lh[x, b - h(x) + 31]
        gp = outer.enter_context(tc.tile_pool(name="gp", bufs=1))
        G = gp.tile([64, NH * HW], F32R, tag="G", name="G")
        q4 = q_all.rearrange("p (n h w) -> p n h w", n=NH, h=H, w=W)
        G4 = G.rearrange("p (n h w) -> p n h w", n=NH, h=H, w=W)
        with tc.tile_pool(name="ps_g", bufs=4,
                          space=bass.MemorySpace.PSUM) as ps_g:
            for ww in range(W):
                ps = ps_g.tile([32, NH * H], F32, tag="g", name=f"gw{ww}")
                nc.tensor.matmul(ps[:], relwT[:, 31 - ww:63 - ww],
                                 q4[:, :, :, ww], start=True, stop=True)
                nc.vector.tensor_copy(G4[0:32, :, :, ww], ps[:])
            for hh in range(H):
                ps = ps_g.tile([32, NH * W], F32, tag="g", name=f"gh{hh}")
                nc.tensor.matmul(ps[:], relhT[:, 31 - hh:63 - hh],
                                 q4[:, :, hh, :], start=True, stop=True)
                nc.vector.tensor_copy(G4[32:64, :, hh, :], ps[:])

        # ---- attention -------------------------------------------------
        ep = outer.enter_context(tc.tile_pool(name="ep", bufs=18))
        zp = outer.enter_context(tc.tile_pool(name="zp", bufs=2))
        op = outer.enter_context(tc.tile_pool(name="op", bufs=4))
        ps_l = outer.enter_context(
            tc.tile_pool(name="ps_l", bufs=3, space=bass.MemorySpace.PSUM))
        ps_o = outer.enter_context(
            tc.tile_pool(name="ps_o", bufs=2, space=bass.MemorySpace.PSUM))
        ps_z = outer.enter_context(
            tc.tile_pool(name="ps_z", bufs=2, space=bass.MemorySpace.PSUM))

        for n in range(NH):
            E = {}
            for j in range(8):
                for ch in range(2):
                    ps = ps_l.tile([128, 512], F32, tag="l",
                                   name=f"l{n}_{j}_{ch}")
                    nc.tensor.matmul(
                        ps[:],
                        k_all[:, n * HW + j * 128:n * HW + (j + 1) * 128],
                        q_all[:, n * HW + ch * 512:n * HW + (ch + 1) * 512],
                        start=True, stop=False)
                    nc.tensor.matmul(
                        ps[:],
                        onehot[:, j * 128:(j + 1) * 128],
                        G[:, n * HW + ch * 512:n * HW + (ch + 1) * 512],
                        start=False, stop=True)
                    e = ep.tile([128, 512], F32R, tag="e", name=f"e{n}_{j}_{ch}")
                    nc.scalar.activation(e[:], ps[:],
                                         mybir.ActivationFunctionType.Exp)
                    E[(j, ch)] = e

            for ch in range(2):
                pso = ps_o.tile([128, 512], F32, tag="o", name=f"o{n}_{ch}")
                psz = ps_z.tile([128, 512], F32, tag="zz", name=f"zz{n}_{ch}")
                for j in range(8):
                    nc.tensor.matmul(
                        pso[:],
                        v_all[:, j * HW + n * 128:j * HW + (n + 1) * 128],
                        E[(j, ch)][:],
                        start=(j == 0), stop=(j == 7))
                    # all-ones stationary -> every PSUM row holds Z[x]
                    nc.tensor.matmul(
                        psz[:], ones[:], E[(j, ch)][:],
                        start=(j == 0), stop=(j == 7))
                rz = zp.tile([128, 512], F32, tag="rz", name=f"rz{n}_{ch}")
                nc.vector.reciprocal(rz[:], psz[:])
                osb = op.tile([128, 512], F32, tag="o", name=f"osb{n}_{ch}")
                nc.vector.tensor_mul(osb[:], pso[:], rz[:])
                nc.sync.dma_start(
                    out_d[n * 128:(n + 1) * 128, ch * 512:(ch + 1) * 512],
                    osb[:])


def _consts():
    ident = np.eye(128, dtype=np.float32)
    onehot = np.zeros((64, HW), np.float32)
    x = np.arange(HW)
    yH, yW = np.divmod(x, W)
    onehot[yW, x] = 1.0
    onehot[32 + yH, x] = 1.0
    ones = np.ones((128, 128), np.float32)
    return ident, onehot, ones


def make_in_maps(featuremap, w_qkv, rel_height, rel_width):
    ident, onehot, ones = _consts()
    w = np.ascontiguousarray(w_qkv, dtype=np.float32)
    rh = np.zeros((2 * H, D), np.float32)
    rh[:2 * H - 1] = rel_height
    rw = np.zeros((2 * W, D), np.float32)
    rw[:2 * W - 1] = rel_width
    maps = []
    for b in range(B):
        maps.append({
            "f": np.ascontiguousarray(
                featuremap[b].reshape(C, HW), dtype=np.float32),
            "w": w, "relh": rh, "relw": rw,
            "ident": ident, "onehot": onehot, "ones": ones,
        })
    return maps


_NC_CACHE = {}


def get_nc():
    if "nc" not in _NC_CACHE:
        _NC_CACHE["nc"] = build_nc()
    return _NC_CACHE["nc"]


def kernel(featuremap, w_qkv, rel_height, rel_width):
    from concourse.bass_utils import run_bass_kernel_spmd

    nc = get_nc()
    in_maps = make_in_maps(featuremap, w_qkv, rel_height, rel_width)
    res = run_bass_kernel_spmd(nc, in_maps, list(range(B)))
    out = np.stack([res.results[b]["out"] for b in range(B)])
    return out.reshape(B, NH * D, H, W)


if __name__ == "__main__":
    nc = build_nc()
    print("built ok:", len(nc.m.functions[0].blocks), "blocks")


# revision 16
# speedup vs baseline: 267.6877x; 267.6877x over previous
"""Trainium2 Bass kernel for 2D MHSA with relative position logits.

Problem (per batch element b of 8, one NeuronCore each — pure data parallel):
    qkv = w_qkv @ featuremap[b]            # [3072, 1024]
    per head n (8 heads, d=128):
      logits = (q*s) @ k^T + relpos(q*s)   # [1024, 1024]
      out[n] = softmax(logits) @ v         # [1024, 128]

Layout strategy (everything chains without transposes after the initial
on-device transpose of w_qkv):
  - q, k produced as [d, x] tiles (d = partitions), v produced transposed
    as [y, d] tiles by swapping matmul operand roles in the projection.
  - logits computed transposed [y, x]; softmax denominator via a ones-
    matmul; 1/Z folded into the output eviction; O^T = v.T-accum directly
    matches the required output layout [n*d, h*w].
  - rel-pos logits: rw[x,(H,W)] = Lw[x, W-w(x)+31] (indep of H), rh
    likewise. Gather matrices G[b,x] = L[x, b-w(x)+31] are built with 64
    shifted-slice matmuls against relT, then folded into the logits PSUM
    accumulation as one K=64 matmul against a constant one-hot matrix.
  - exp() happens on the Scalar engine during PSUM eviction; no max
    subtraction (logits are bounded by ~±2 here; validated vs reference).

All matmul operands are float32r (full fp32 storage; PE fast mode).
"""

import os
import sys

for _p in ("/opt/trn_rl_repo", "/root/.axon_site/_ro/trn_rl_repo"):
    if os.path.isdir(_p) and _p not in sys.path:
        sys.path.append(_p)

import numpy as np

import concourse.bass as bass
import concourse.tile as tile
from concourse import bacc, mybir

F32R = mybir.dt.float32r
F32 = mybir.dt.float32
BF16 = mybir.dt.bfloat16

B = 8          # batch == number of cores
NH = 8         # heads
D = 128        # head dim
H = 32
W = 32
HW = H * W     # 1024 positions
C = 512        # channels
O3 = 3 * NH * D  # 3072 qkv rows
SCALE = D ** -0.5


def build_nc(num_devices: int = B):
    nc = bacc.Bacc("TRN2", target_bir_lowering=False, debug=False,
                   num_devices=num_devices)

    f_d = nc.dram_tensor("f", [C, HW], F32R, kind="ExternalInput")
    w_d = nc.dram_tensor("w", [O3, C], F32R, kind="ExternalInput")
    relh_d = nc.dram_tensor("relh", [2 * H, D], F32R, kind="ExternalInput")
    relw_d = nc.dram_tensor("relw", [2 * W, D], F32R, kind="ExternalInput")
    ident_d = nc.dram_tensor("ident", [128, 128], F32R, kind="ExternalInput")
    onehot_d = nc.dram_tensor("onehot", [64, HW], BF16, kind="ExternalInput")
    ones_d = nc.dram_tensor("ones", [128, 128], F32R, kind="ExternalInput")
    out_d = nc.dram_tensor("out", [NH * D, HW], F32, kind="ExternalOutput")

    bench_loop = int(os.environ.get("BENCH_LOOP", "0"))
    with tile.TileContext(nc) as tc:
        if bench_loop > 1:
            with tc.For_i(0, bench_loop, 1):
                _trace(nc, tc, f_d, w_d, relh_d, relw_d, ident_d, onehot_d,
                       ones_d, out_d)
        else:
            _trace(nc, tc, f_d, w_d, relh_d, relw_d, ident_d, onehot_d,
                   ones_d, out_d)
    nc.compile()
    return nc


def _trace(nc, tc, f_d, w_d, relh_d, relw_d, ident_d, onehot_d, ones_d, out_d):
    from contextlib import ExitStack

    with ExitStack() as outer:
        # ---- persistent SBUF pools -------------------------------------
        big = outer.enter_context(tc.tile_pool(name="big", bufs=1))
        q_all = big.tile([128, NH * HW], BF16, tag="q_all", name="q_all")
        k_all = big.tile([128, NH * HW], BF16, tag="k_all", name="k_all")
        v_all = big.tile([128, NH * HW], F32R, tag="v_all", name="v_all")

        cst = outer.enter_context(tc.tile_pool(name="cst", bufs=1))
        ident = cst.tile([128, 128], F32R, tag="ident", name="ident")
        onehot = cst.tile([64, HW], BF16, tag="onehot", name="onehot")
        ones = cst.tile([128, 128], F32R, tag="ones", name="ones")
        relwT = cst.tile([128, 64], BF16, tag="relwT", name="relwT")
        relhT = cst.tile([128, 64], BF16, tag="relhT", name="relhT")

        nc.sync.dma_start(ident[:], ident_d[:])
        nc.sync.dma_start(onehot[:], onehot_d[:])
        nc.sync.dma_start(ones[:], ones_d[:])

        # ---- phase 1: transpose w, project q/k/v -----------------------
        with ExitStack() as ph1:
            fp = ph1.enter_context(tc.tile_pool(name="fp", bufs=1))
            wtp = ph1.enter_context(tc.tile_pool(name="wtp", bufs=1))
            wst = ph1.enter_context(tc.tile_pool(name="wst", bufs=8))
            rst = ph1.enter_context(tc.tile_pool(name="rst", bufs=2))
            ps_tr = ph1.enter_context(
                tc.tile_pool(name="ps_tr", bufs=2, space=bass.MemorySpace.PSUM))
            ps_pj = ph1.enter_context(
                tc.tile_pool(name="ps_pj", bufs=4, space=bass.MemorySpace.PSUM))

            f_sb = []
            for i in range(4):
                ft = fp.tile([128, HW], F32R, tag=f"f{i}", name=f"f{i}")
                nc.sync.dma_start(ft[:], f_d[i * 128:(i + 1) * 128, :])
                f_sb.append(ft)

            # rel transposes: [63, 128] -> [128, 63] (padded to 64 rows for
            # the f32r transpose mode; col 63 of relT is never read)
            for (rd, rT, nm) in ((relw_d, relwT, "rw"), (relh_d, relhT, "rh")):
                rs = rst.tile([64, 128], F32R, tag="rs", name=f"rs_{nm}")
                nc.sync.dma_start(rs[:], rd[:])
                pt = ps_tr.tile([128, 64], F32R, tag="ptr_r", name=f"ptr_{nm}")
                nc.tensor.transpose(pt[:], rs[:], ident[0:64, 0:64])
                nc.vector.tensor_copy(rT[:], pt[:].bitcast(F32))

            # w transpose: wT[cb] [128, 3072], wT[cb][:, o] = w[o, cb*128+p]
            wT = []
            for cb in range(4):
                t = wtp.tile([128, O3], F32R, tag=f"wT{cb}", name=f"wT{cb}")
                wT.append(t)
            for og in range(6):
                wtiles = []
                for oi in range(4):
                    ob = og * 4 + oi
                    wt_ = wst.tile([128, C], F32R, tag="wst", name=f"w{ob}")
                    nc.sync.dma_start(wt_[:], w_d[ob * 128:(ob + 1) * 128, :])
                    wtiles.append(wt_)
                for cb in range(4):
                    pt = ps_tr.tile([128, 512], F32R, tag="ptr_w",
                                    name=f"ptw{og}_{cb}")
                    for oi in range(4):
                        nc.tensor.transpose(
                            pt[:, oi * 128:(oi + 1) * 128],
                            wtiles[oi][:, cb * 128:(cb + 1) * 128],
                            ident[:])
                    nc.vector.tensor_copy(
                        wT[cb][:, og * 512:(og + 1) * 512], pt[:])

            # q/k projection: out[o_blk(128), x] ; o_blk 0-7 q heads, 8-15 k
            for ob in range(16):
                for ch in range(2):
                    ps = ps_pj.tile([128, 512], F32, tag="pj", name=f"pjqk{ob}_{ch}")
                    for cb in range(4):
                        nc.tensor.matmul(
                            ps[:],
                            wT[cb][:, ob * 128:(ob + 1) * 128],
                            f_sb[cb][:, ch * 512:(ch + 1) * 512],
                            start=(cb == 0), stop=(cb == 3))
                    dst = (q_all if ob < 8 else k_all)
                    col = (ob % 8) * HW + ch * 512
                    if ob < 8:
                        nc.vector.tensor_scalar_mul(
                            dst[:, col:col + 512], ps[:], SCALE)
                    else:
                        nc.vector.tensor_copy(dst[:, col:col + 512], ps[:])

            # v projection, transposed: out[y_blk(128), o_v] with
            # lhsT = f tile, rhs = wT v-columns
            for yb in range(8):
                for oc in range(2):
                    ps = ps_pj.tile([128, 512], F32, tag="pj", name=f"pjv{yb}_{oc}")
                    for cb in range(4):
                        nc.tensor.matmul(
                            ps[:],
                            f_sb[cb][:, yb * 128:(yb + 1) * 128],
                            wT[cb][:, 2048 + oc * 512:2048 + (oc + 1) * 512],
                            start=(cb == 0), stop=(cb == 3))
                    nc.vector.tensor_copy(
                        v_all[:, yb * HW + oc * 512:yb * HW + (oc + 1) * 512],
                        ps[:])

        # ---- G gather matrices -----------------------------------------
        # G[b, x] (b<32):  Lw[x, b - w(x) + 31] ; G[32+b, x]: Lh[x, b - h(x) + 31]
        gp = outer.enter_context(tc.tile_pool(name="gp", bufs=1))
        G = gp.tile([64, NH * HW], BF16, tag="G", name="G")
        q4 = q_all.rearrange("p (n h w) -> p n h w", n=NH, h=H, w=W)
        G4 = G.rearrange("p (n h w) -> p n h w", n=NH, h=H, w=W)
        with tc.tile_pool(name="ps_g", bufs=4,
                          space=bass.MemorySpace.PSUM) as ps_g:
            for ww in range(W):
                ps = ps_g.tile([32, NH * H], F32, tag="g", name=f"gw{ww}")
                nc.tensor.matmul(ps[:], relwT[:, 31 - ww:63 - ww],
                                 q4[:, :, :, ww], start=True, stop=True)
                nc.vector.tensor_copy(G4[0:32, :, :, ww], ps[:])
            for hh in range(H):
                ps = ps_g.tile([32, NH * W], F32, tag="g", name=f"gh{hh}")
                nc.tensor.matmul(ps[:], relhT[:, 31 - hh:63 - hh],
                                 q4[:, :, hh, :], start=True, stop=True)
                nc.vector.tensor_copy(G4[32:64, :, hh, :], ps[:])

        # ---- attention -------------------------------------------------
        ep = outer.enter_context(tc.tile_pool(name="ep", bufs=11))
        zp = outer.enter_context(tc.tile_pool(name="zp", bufs=2))
        op = outer.enter_context(tc.tile_pool(name="op", bufs=6))
        ps_l = outer.enter_context(
            tc.tile_pool(name="ps_l", bufs=2, space=bass.MemorySpace.PSUM))
        ps_o = outer.enter_context(
            tc.tile_pool(name="ps_o", bufs=2, space=bass.MemorySpace.PSUM))
        ps_z = outer.enter_context(
            tc.tile_pool(name="ps_z", bufs=2, space=bass.MemorySpace.PSUM))

        for n in range(NH):
            E = {}
            for j in range(8):
                ps = ps_l.tile([128, 1024], F32, tag="l", name=f"l{n}_{j}")
                for ch in range(2):
                    sl = slice(ch * 512, (ch + 1) * 512)
                    nc.tensor.matmul(
                        ps[:, sl],
                        k_all[:, n * HW + j * 128:n * HW + (j + 1) * 128],
                        q_all[:, n * HW + ch * 512:n * HW + (ch + 1) * 512],
                        start=True, stop=False)
                    nc.tensor.matmul(
                        ps[:, sl],
                        onehot[:, j * 128:(j + 1) * 128],
                        G[:, n * HW + ch * 512:n * HW + (ch + 1) * 512],
                        start=False, stop=True)
                e = ep.tile([128, 1024], F32R, tag="e", name=f"e{n}_{j}")
                nc.scalar.activation(e[:], ps[:],
                                     mybir.ActivationFunctionType.Exp)
                E[j] = e

            for ch in range(2):
                pso = ps_o.tile([128, 512], F32, tag="o", name=f"o{n}_{ch}")
                psz = ps_z.tile([128, 512], F32, tag="zz", name=f"zz{n}_{ch}")
                for j in range(8):
                    esl = E[j][:, ch * 512:(ch + 1) * 512]
                    nc.tensor.matmul(
                        pso[:],
                        v_all[:, j * HW + n * 128:j * HW + (n + 1) * 128],
                        esl, start=(j == 0), stop=(j == 7))
                    # all-ones stationary -> every PSUM row holds Z[x]
                    nc.tensor.matmul(
                        psz[:], ones[:], esl,
                        start=(j == 0), stop=(j == 7))
                rz = zp.tile([128, 512], F32, tag="rz", name=f"rz{n}_{ch}")
                nc.vector.reciprocal(rz[:], psz[:])
                osb = op.tile([128, 512], F32, tag="o", name=f"osb{n}_{ch}")
                nc.vector.tensor_mul(osb[:], pso[:], rz[:])
                nc.sync.dma_start(
                    out_d[n * 128:(n + 1) * 128, ch * 512:(ch + 1) * 512],
                    osb[:])


def _consts():
    ident = np.eye(128, dtype=np.float32)
    onehot = np.zeros((64, HW), np.float32)
    x = np.arange(HW)
    yH, yW = np.divmod(x, W)
    onehot[yW, x] = 1.0
    onehot[32 + yH, x] = 1.0
    ones = np.ones((128, 128), np.float32)
    import ml_dtypes
    return ident, onehot.astype(ml_dtypes.bfloat16), ones


def make_in_maps(featuremap, w_qkv, rel_height, rel_width):
    ident, onehot, ones = _consts()
    w = np.ascontiguousarray(w_qkv, dtype=np.float32)
    rh = np.zeros((2 * H, D), np.float32)
    rh[:2 * H - 1] = rel_height
    rw = np.zeros((2 * W, D), np.float32)
    rw[:2 * W - 1] = rel_width
    maps = []
    for b in range(B):
        maps.append({
            "f": np.ascontiguousarray(
                featuremap[b].reshape(C, HW), dtype=np.float32),
            "w": w, "relh": rh, "relw": rw,
            "ident": ident, "onehot": onehot, "ones": ones,
        })
    return maps


_NC_CACHE = {}


def get_nc():
    if "nc" not in _NC_CACHE:
        _NC_CACHE["nc"] = build_nc()
    return _NC_CACHE["nc"]


def kernel(featuremap, w_qkv, rel_height, rel_width):
    from concourse.bass_utils import run_bass_kernel_spmd

    nc = get_nc()
    in_maps = make_in_maps(featuremap, w_qkv, rel_height, rel_width)
    res = run_bass_kernel_spmd(nc, in_maps, list(range(B)))
    out = np.stack([res.results[b]["out"] for b in range(B)])
    return out.reshape(B, NH * D, H, W)


if __name__ == "__main__":
    nc = build_nc()
    print("built ok:", len(nc.m.functions[0].blocks), "blocks")


# revision 21
# speedup vs baseline: 277.0598x; 1.0350x over previous
"""Trainium2 Bass kernel for 2D MHSA with relative position logits.

Problem (per batch element b of 8, one NeuronCore each — pure data parallel):
    qkv = w_qkv @ featuremap[b]            # [3072, 1024]
    per head n (8 heads, d=128):
      logits = (q*s) @ k^T + relpos(q*s)   # [1024, 1024]
      out[n] = softmax(logits) @ v         # [1024, 128]

Layout strategy (everything chains without transposes after the initial
on-device transpose of w_qkv):
  - q, k produced as [d, x] tiles (d = partitions), v produced transposed
    as [y, d] tiles by swapping matmul operand roles in the projection.
  - logits computed transposed [y, x]; softmax denominator via a ones-
    matmul; 1/Z folded into the output eviction; O^T = v.T-accum directly
    matches the required output layout [n*d, h*w].
  - rel-pos logits: rw[x,(H,W)] = Lw[x, W-w(x)+31] (indep of H), rh
    likewise. Gather matrices G[b,x] = L[x, b-w(x)+31] are built with 64
    shifted-slice matmuls against relT, then folded into the logits PSUM
    accumulation as one K=64 matmul against a constant one-hot matrix.
  - exp() happens on the Scalar engine during PSUM eviction; no max
    subtraction (logits are bounded by ~±2 here; validated vs reference).

All matmul operands are float32r (full fp32 storage; PE fast mode).
"""

import os
import sys

for _p in ("/opt/trn_rl_repo", "/root/.axon_site/_ro/trn_rl_repo"):
    if os.path.isdir(_p) and _p not in sys.path:
        sys.path.append(_p)

import numpy as np

import concourse.bass as bass
import concourse.tile as tile
from concourse import bacc, mybir

F32R = mybir.dt.float32r
F32 = mybir.dt.float32
BF16 = mybir.dt.bfloat16

B = 8          # batch == number of cores
NH = 8         # heads
D = 128        # head dim
H = 32
W = 32
HW = H * W     # 1024 positions
C = 512        # channels
O3 = 3 * NH * D  # 3072 qkv rows
SCALE = D ** -0.5


def build_nc(num_devices: int = B):
    nc = bacc.Bacc("TRN2", target_bir_lowering=False, debug=False,
                   num_devices=num_devices)

    f_d = nc.dram_tensor("f", [C, HW], F32R, kind="ExternalInput")
    w_d = nc.dram_tensor("w", [O3, C], F32R, kind="ExternalInput")
    relh_d = nc.dram_tensor("relh", [2 * H, D], F32R, kind="ExternalInput")
    relw_d = nc.dram_tensor("relw", [2 * W, D], F32R, kind="ExternalInput")
    ident_d = nc.dram_tensor("ident", [128, 128], F32R, kind="ExternalInput")
    onehot_d = nc.dram_tensor("onehot", [64, HW], BF16, kind="ExternalInput")
    ones_d = nc.dram_tensor("ones", [128, 128], BF16, kind="ExternalInput")
    out_d = nc.dram_tensor("out", [NH * D, HW], F32, kind="ExternalOutput")

    bench_loop = int(os.environ.get("BENCH_LOOP", "0"))
    with tile.TileContext(nc) as tc:
        if bench_loop > 1:
            with tc.For_i(0, bench_loop, 1):
                _trace(nc, tc, f_d, w_d, relh_d, relw_d, ident_d, onehot_d,
                       ones_d, out_d)
        else:
            _trace(nc, tc, f_d, w_d, relh_d, relw_d, ident_d, onehot_d,
                   ones_d, out_d)
    nc.compile()
    return nc


def _trace(nc, tc, f_d, w_d, relh_d, relw_d, ident_d, onehot_d, ones_d, out_d):
    from contextlib import ExitStack

    with ExitStack() as outer:
        # ---- persistent SBUF pools -------------------------------------
        big = outer.enter_context(tc.tile_pool(name="big", bufs=1))
        q_all = big.tile([128, NH * HW], BF16, tag="q_all", name="q_all")
        k_all = big.tile([128, NH * HW], BF16, tag="k_all", name="k_all")
        v_all = big.tile([128, NH * HW], BF16, tag="v_all", name="v_all")

        cst = outer.enter_context(tc.tile_pool(name="cst", bufs=1))
        ident = cst.tile([128, 128], F32R, tag="ident", name="ident")
        onehot = cst.tile([64, HW], BF16, tag="onehot", name="onehot")
        ones = cst.tile([128, 128], BF16, tag="ones", name="ones")
        relwT = cst.tile([128, 64], BF16, tag="relwT", name="relwT")
        relhT = cst.tile([128, 64], BF16, tag="relhT", name="relhT")

        nc.sync.dma_start(ident[:], ident_d[:])
        nc.sync.dma_start(onehot[:], onehot_d[:])
        nc.sync.dma_start(ones[:], ones_d[:])

        # ---- phase 1: transpose w, project q/k/v -----------------------
        with ExitStack() as ph1:
            fp = ph1.enter_context(tc.tile_pool(name="fp", bufs=1))
            wtp = ph1.enter_context(tc.tile_pool(name="wtp", bufs=1))
            wst = ph1.enter_context(tc.tile_pool(name="wst", bufs=8))
            rst = ph1.enter_context(tc.tile_pool(name="rst", bufs=2))
            ps_tr = ph1.enter_context(
                tc.tile_pool(name="ps_tr", bufs=2, space=bass.MemorySpace.PSUM))
            ps_pj = ph1.enter_context(
                tc.tile_pool(name="ps_pj", bufs=4, space=bass.MemorySpace.PSUM))

            f_sb = []
            for i in range(4):
                ft = fp.tile([128, HW], F32R, tag=f"f{i}", name=f"f{i}")
                nc.sync.dma_start(ft[:], f_d[i * 128:(i + 1) * 128, :])
                f_sb.append(ft)

            # rel transposes: [63, 128] -> [128, 63] (padded to 64 rows for
            # the f32r transpose mode; col 63 of relT is never read)
            for (rd, rT, nm) in ((relw_d, relwT, "rw"), (relh_d, relhT, "rh")):
                rs = rst.tile([64, 128], F32R, tag="rs", name=f"rs_{nm}")
                nc.sync.dma_start(rs[:], rd[:])
                pt = ps_tr.tile([128, 64], F32R, tag="ptr_r", name=f"ptr_{nm}")
                nc.tensor.transpose(pt[:], rs[:], ident[0:64, 0:64])
                nc.vector.tensor_copy(rT[:], pt[:].bitcast(F32))

            # w transpose: wT[cb] [128, 3072], wT[cb][:, o] = w[o, cb*128+p]
            wT = []
            for cb in range(4):
                t = wtp.tile([128, O3], F32R, tag=f"wT{cb}", name=f"wT{cb}")
                wT.append(t)
            for og in range(6):
                wtiles = []
                for oi in range(4):
                    ob = og * 4 + oi
                    wt_ = wst.tile([128, C], F32R, tag="wst", name=f"w{ob}")
                    nc.sync.dma_start(wt_[:], w_d[ob * 128:(ob + 1) * 128, :])
                    wtiles.append(wt_)
                for cb in range(4):
                    pt = ps_tr.tile([128, 512], F32R, tag="ptr_w",
                                    name=f"ptw{og}_{cb}")
                    for oi in range(4):
                        nc.tensor.transpose(
                            pt[:, oi * 128:(oi + 1) * 128],
                            wtiles[oi][:, cb * 128:(cb + 1) * 128],
                            ident[:])
                    nc.vector.tensor_copy(
                        wT[cb][:, og * 512:(og + 1) * 512], pt[:])

            # q/k projection: out[o_blk(128), x] ; o_blk 0-7 q heads, 8-15 k
            for ob in range(16):
                for ch in range(2):
                    ps = ps_pj.tile([128, 512], F32, tag="pj", name=f"pjqk{ob}_{ch}")
                    for cb in range(4):
                        nc.tensor.matmul(
                            ps[:],
                            wT[cb][:, ob * 128:(ob + 1) * 128],
                            f_sb[cb][:, ch * 512:(ch + 1) * 512],
                            start=(cb == 0), stop=(cb == 3))
                    dst = (q_all if ob < 8 else k_all)
                    col = (ob % 8) * HW + ch * 512
                    if ob < 8:
                        nc.vector.tensor_scalar_mul(
                            dst[:, col:col + 512], ps[:], SCALE)
                    else:
                        nc.vector.tensor_copy(dst[:, col:col + 512], ps[:])

            # v projection, transposed: out[y_blk(128), o_v] with
            # lhsT = f tile, rhs = wT v-columns
            for yb in range(8):
                for oc in range(2):
                    ps = ps_pj.tile([128, 512], F32, tag="pj", name=f"pjv{yb}_{oc}")
                    for cb in range(4):
                        nc.tensor.matmul(
                            ps[:],
                            f_sb[cb][:, yb * 128:(yb + 1) * 128],
                            wT[cb][:, 2048 + oc * 512:2048 + (oc + 1) * 512],
                            start=(cb == 0), stop=(cb == 3))
                    nc.vector.tensor_copy(
                        v_all[:, yb * HW + oc * 512:yb * HW + (oc + 1) * 512],
                        ps[:])

        # ---- G gather matrices -----------------------------------------
        # G[b, x] (b<32):  Lw[x, b - w(x) + 31] ; G[32+b, x]: Lh[x, b - h(x) + 31]
        gp = outer.enter_context(tc.tile_pool(name="gp", bufs=1))
        G = gp.tile([64, NH * HW], BF16, tag="G", name="G")
        q4 = q_all.rearrange("p (n h w) -> p n h w", n=NH, h=H, w=W)
        G4 = G.rearrange("p (n h w) -> p n h w", n=NH, h=H, w=W)
        with tc.tile_pool(name="ps_g", bufs=4,
                          space=bass.MemorySpace.PSUM) as ps_g:
            for ww in range(W):
                ps = ps_g.tile([32, NH * H], F32, tag="g", name=f"gw{ww}")
                nc.tensor.matmul(ps[:], relwT[:, 31 - ww:63 - ww],
                                 q4[:, :, :, ww], start=True, stop=True)
                nc.vector.tensor_copy(G4[0:32, :, :, ww], ps[:])
            for hh in range(H):
                ps = ps_g.tile([32, NH * W], F32, tag="g", name=f"gh{hh}")
                nc.tensor.matmul(ps[:], relhT[:, 31 - hh:63 - hh],
                                 q4[:, :, hh, :], start=True, stop=True)
                nc.vector.tensor_copy(G4[32:64, :, hh, :], ps[:])

        # ---- attention -------------------------------------------------
        ep = outer.enter_context(tc.tile_pool(name="ep", bufs=11))
        zp = outer.enter_context(tc.tile_pool(name="zp", bufs=2))
        op = outer.enter_context(tc.tile_pool(name="op", bufs=6))
        ps_l = outer.enter_context(
            tc.tile_pool(name="ps_l", bufs=2, space=bass.MemorySpace.PSUM))
        ps_o = outer.enter_context(
            tc.tile_pool(name="ps_o", bufs=2, space=bass.MemorySpace.PSUM))
        ps_z = outer.enter_context(
            tc.tile_pool(name="ps_z", bufs=2, space=bass.MemorySpace.PSUM))

        for n in range(NH):
            E = {}
            for j in range(8):
                ps = ps_l.tile([128, 1024], F32, tag="l", name=f"l{n}_{j}")
                for ch in range(2):
                    sl = slice(ch * 512, (ch + 1) * 512)
                    nc.tensor.matmul(
                        ps[:, sl],
                        k_all[:, n * HW + j * 128:n * HW + (j + 1) * 128],
                        q_all[:, n * HW + ch * 512:n * HW + (ch + 1) * 512],
                        start=True, stop=False)
                    nc.tensor.matmul(
                        ps[:, sl],
                        onehot[:, j * 128:(j + 1) * 128],
                        G[:, n * HW + ch * 512:n * HW + (ch + 1) * 512],
                        start=False, stop=True)
                e = ep.tile([128, 1024], BF16, tag="e", name=f"e{n}_{j}")
                nc.scalar.activation(e[:], ps[:],
                                     mybir.ActivationFunctionType.Exp)
                E[j] = e

            for ch in range(2):
                pso = ps_o.tile([128, 512], F32, tag="o", name=f"o{n}_{ch}")
                psz = ps_z.tile([128, 512], F32, tag="zz", name=f"zz{n}_{ch}")
                for j in range(8):
                    esl = E[j][:, ch * 512:(ch + 1) * 512]
                    nc.tensor.matmul(
                        pso[:],
                        v_all[:, j * HW + n * 128:j * HW + (n + 1) * 128],
                        esl, start=(j == 0), stop=(j == 7))
                    # all-ones stationary -> every PSUM row holds Z[x]
                    nc.tensor.matmul(
                        psz[:], ones[:], esl,
                        start=(j == 0), stop=(j == 7))
                rz = zp.tile([128, 512], F32, tag="rz", name=f"rz{n}_{ch}")
                nc.vector.reciprocal(rz[:], psz[:])
                osb = op.tile([128, 512], F32, tag="o", name=f"osb{n}_{ch}")
                nc.vector.tensor_mul(osb[:], pso[:], rz[:])
                nc.sync.dma_start(
                    out_d[n * 128:(n + 1) * 128, ch * 512:(ch + 1) * 512],
                    osb[:])


def _consts():
    ident = np.eye(128, dtype=np.float32)
    onehot = np.zeros((64, HW), np.float32)
    x = np.arange(HW)
    yH, yW = np.divmod(x, W)
    onehot[yW, x] = 1.0
    onehot[32 + yH, x] = 1.0
    ones = np.ones((128, 128), np.float32)
    import ml_dtypes
    return ident, onehot.astype(ml_dtypes.bfloat16), ones.astype(ml_dtypes.bfloat16)


def make_in_maps(featuremap, w_qkv, rel_height, rel_width):
    ident, onehot, ones = _consts()
    w = np.ascontiguousarray(w_qkv, dtype=np.float32)
    rh = np.zeros((2 * H, D), np.float32)
    rh[:2 * H - 1] = rel_height
    rw = np.zeros((2 * W, D), np.float32)
    rw[:2 * W - 1] = rel_width
    maps = []
    for b in range(B):
        maps.append({
            "f": np.ascontiguousarray(
                featuremap[b].reshape(C, HW), dtype=np.float32),
            "w": w, "relh": rh, "relw": rw,
            "ident": ident, "onehot": onehot, "ones": ones,
        })
    return maps


_NC_CACHE = {}


def get_nc():
    if "nc" not in _NC_CACHE:
        _NC_CACHE["nc"] = build_nc()
    return _NC_CACHE["nc"]


def kernel(featuremap, w_qkv, rel_height, rel_width):
    from concourse.bass_utils import run_bass_kernel_spmd

    nc = get_nc()
    in_maps = make_in_maps(featuremap, w_qkv, rel_height, rel_width)
    res = run_bass_kernel_spmd(nc, in_maps, list(range(B)))
    out = np.stack([res.results[b]["out"] for b in range(B)])
    return out.reshape(B, NH * D, H, W)


if __name__ == "__main__":
    nc = build_nc()
    print("built ok:", len(nc.m.functions[0].blocks), "blocks")
